# revision 25
# baseline (speedup 1.0000x reference)
"""Causal multi-head attention on 8 TRN2 NeuronCores.

Sharding: core c -> (batch b = c // 2, head-half hh = c % 2).
Each core computes QKV for its 8 heads over the full sequence of its batch,
causal flash attention, and a partial out-projection using its 512 rows of
w_out. The host sums the two partials per batch (the "all-reduce" of the
tensor-parallel out projection).

Fully fused single-stream schedule: the attention iterations of q-chunk c
interleave "filler" PE jobs — the K/V/Q projections of chunk c+1 and the
out-projection of chunk c-1 — so the tensor engine never drains at chunk
boundaries and the ACT-bound attention stretches stay packed with PE work.

All matmul operands are bf16 (same PE rate as fp32r, half the DMA/SBUF
traffic; measured end-to-end rel err ~5.6e-3 vs 2e-2 budget). PSUM fp32.

Layouts (per core):
  KT[j]  [128, 2048] bf16  K^T, head pair j (head 2j rows 0:64, 2j+1 64:128)
  V[t]   [128, 520]  bf16  V token-tile t, 8 heads x (64 cols + ones col)
                           for the softmax denominator; ones via memset
  QT[j]  [128, 512]  bf16  per-q-chunk Q^T, same row split; S matmuls
                           contract K=64 on the row halves

Shapes (hardcoded): B=4, T=2048, D=1024, H=16, HD=64.
"""
import sys

for _p in ('/opt/trn_rl_repo', '/root/.axon_site/_ro/trn_rl_repo'):
    if _p not in sys.path:
        sys.path.insert(0, _p)

import numpy as np

B, T, D = 4, 2048, 1024
H, HD = 16, 64
HPC = H // 2          # heads per core = 8
DPC = HPC * HD        # out-dims per core = 512
N_CORES = 8

_nc_cache = {}


def _build_nc():
    import concourse.bacc as bacc
    import concourse.mybir as mybir
    from concourse.tile import TileContext

    F32 = mybir.dt.float32
    BF16 = mybir.dt.bfloat16
    AF = mybir.ActivationFunctionType
    ALU = mybir.AluOpType

    CH = 512              # token chunk (== QC)
    QC = 512              # query chunk
    NKB = T // 128        # 16 k-blocks
    NQC = T // QC         # 4 query chunks
    NDT = D // 128        # 8 input-dim tiles
    VW = HPC * (HD + 1)   # V tile width = 520

    nc = bacc.Bacc('TRN2', target_bir_lowering=False, debug=False)
    xT_d = nc.dram_tensor('xT', [D, T], BF16, kind='ExternalInput')
    # wq/wv/wo ship pre-packed as [128, d, cols] so each loads in ONE DMA
    # (SP-engine trigger processing is ~0.6us per DMA and crowds startup)
    wq_d = nc.dram_tensor('wq', [128, NDT * DPC], BF16, kind='ExternalInput')
    wk_d = nc.dram_tensor('wk', [D, DPC], BF16, kind='ExternalInput')
    wv_d = nc.dram_tensor('wv', [128, NDT * DPC], BF16, kind='ExternalInput')
    wo_d = nc.dram_tensor('wo', [128, 4 * D], BF16, kind='ExternalInput')
    po_d = nc.dram_tensor('po', [T, D], BF16, kind='ExternalOutput')

    with nc.allow_low_precision(reason='bf16 matmuls by design'), \
            TileContext(nc) as tc:
        with (
            tc.tile_pool(name='kt', bufs=1) as kt_pool,
            tc.tile_pool(name='vv', bufs=1) as v_pool,
            tc.tile_pool(name='small', bufs=2) as sm_pool,
            tc.tile_pool(name='wgt', bufs=1) as w_pool,
            tc.tile_pool(name='qt', bufs=2) as qt_pool,
            tc.tile_pool(name='xs', bufs=1) as x_pool,
            tc.tile_pool(name='ao', bufs=2) as ao_pool,
            tc.tile_pool(name='pt', bufs=4) as pt_pool,
            tc.tile_pool(name='osb', bufs=4) as osb_pool,
            tc.tile_pool(name='ps_s', bufs=2, space='PSUM') as ps_s,
            tc.tile_pool(name='ps_ot', bufs=2, space='PSUM') as ps_ot,
            tc.tile_pool(name='ps_pp', bufs=2, space='PSUM') as ps_pp,
        ):
            WQb = w_pool.tile([128, NDT * DPC], BF16, tag='wq', name='wqs')
            WQb_r = WQb.rearrange('p (d c) -> p d c', c=DPC)
            WK = [w_pool.tile([128, DPC], BF16, tag=f'wk{d}',
                              name=f'wks{d}') for d in range(NDT)]
            WVb = w_pool.tile([128, NDT * DPC], BF16, tag='wv', name='wvs')
            WVb_r = WVb.rearrange('p (d c) -> p d c', c=DPC)
            WOb = w_pool.tile([128, 4 * D], BF16, tag='wo', name='wos')
            WOb_r = WOb.rearrange('p (d c) -> p d c', c=D)
            WQ = [WQb_r[:, d, :] for d in range(NDT)]
            WV = [WVb_r[:, d, :] for d in range(NDT)]
            WO = [WOb_r[:, d, :] for d in range(4)]

            # pre-warm the ACT exp table so the first real exp doesn't pay
            # the ~2.7us table load
            warm = sm_pool.tile([1, 16], F32, tag='warm', bufs=1)
            warm2 = sm_pool.tile([2, 16], F32, tag='warm2', bufs=1)
            nc.vector.memset(warm[:, :], 0.0)
            nc.scalar.activation(warm[:, :], warm[:, :], AF.Exp)
            nc.gpsimd.affine_select(
                out=warm[:, :], in_=warm[:, :], compare_op=ALU.is_ge,
                fill=0.0, base=0, channel_multiplier=-1, pattern=[[1, 16]])
            nc.gpsimd.partition_broadcast(warm2[:, :], warm[:, :])

            KT = [kt_pool.tile([128, T], BF16, tag=f'kt{j}', name=f'kt{j}')
                  for j in range(4)]
            V = [v_pool.tile([128, VW], BF16, tag=f'v{t}', name=f'v{t}')
                 for t in range(NKB)]

            # x fully resident: one [128, T] tile per input-dim slice, one
            # DMA each; chunk c reads columns [c*CH, (c+1)*CH)
            XF = [x_pool.tile([128, T], BF16, tag=f'x{d}', name=f'xf{d}')
                  for d in range(NDT)]
            xs_sets = {c: [XF[d][:, c*CH:(c+1)*CH] for d in range(NDT)]
                       for c in range(NQC)}
            qt_sets = {}

            def alloc_qt(c):
                qt_sets[c] = [qt_pool.tile([128, QC], BF16, tag=f'qt{j}',
                                           name=f'qt{j}_{c & 1}')
                              for j in range(4)]
                return qt_sets[c]

            # DMA emission order is startup-critical: interleave x with WK
            # so the first K matmuls start after ~2 tiles
            for d in range(NDT):
                nc.sync.dma_start(XF[d][:, :], xT_d[d*128:(d+1)*128, :])
                nc.sync.dma_start(WK[d][:, :], wk_d[d*128:(d+1)*128, :])
            nc.sync.dma_start(WVb[:, :], wv_d[:, :])
            nc.sync.dma_start(WQb[:, :], wq_d[:, :])
            nc.sync.dma_start(WOb[:, :], wo_d[:, :])

            # softmax-denominator ones columns: on-chip (no descriptor spam)
            for t in range(NKB):
                vt3 = V[t].rearrange('p (h c) -> p h c', c=HD + 1)
                nc.gpsimd.memset(vt3[:, :, HD], 1.0)

            def proj_jobs(c):
                """K/V/Q projection of chunk c as 12 single-psum-tile jobs."""
                xs = xs_sets[c]
                QTs = alloc_qt(c)
                jobs = []
                for j in range(4):
                    def kj(j=j, xs=xs, c=c):
                        pp = ps_pp.tile([128, CH], F32, tag='pp',
                                        name=f'ppk{j}')
                        for d in range(NDT):
                            nc.tensor.matmul(
                                pp[:, :],
                                lhsT=WK[d][:, j*128:(j+1)*128],
                                rhs=xs[d][:, :],
                                start=(d == 0), stop=(d == NDT - 1))
                        nc.vector.tensor_copy(
                            KT[j][:, c*CH:(c+1)*CH], pp[:, :])
                    jobs.append(kj)
                for tt in range(4):
                    def vj(tt=tt, xs=xs, c=c):
                        pp = ps_pp.tile([128, DPC], F32, tag='pp',
                                        name=f'ppv{tt}')
                        for d in range(NDT):
                            nc.tensor.matmul(
                                pp[:, :],
                                lhsT=xs[d][:, tt*128:(tt+1)*128],
                                rhs=WV[d][:, :],
                                start=(d == 0), stop=(d == NDT - 1))
                        vt3 = V[c*4 + tt].rearrange('p (h c) -> p h c',
                                                    c=HD + 1)
                        nc.vector.tensor_copy(
                            vt3[:, :, 0:HD],
                            pp.rearrange('p (h c) -> p h c', c=HD))
                    jobs.append(vj)
                for j in range(4):
                    def qj(j=j, xs=xs, QTs=QTs):
                        pp = ps_pp.tile([128, QC], F32, tag='pp',
                                        name=f'ppq{j}')
                        for d in range(NDT):
                            nc.tensor.matmul(
                                pp[:, :],
                                lhsT=WQ[d][:, j*128:(j+1)*128],
                                rhs=xs[d][:, :],
                                start=(d == 0), stop=(d == NDT - 1))
                        nc.vector.tensor_copy(QTs[j][:, :], pp[:, :])
                    jobs.append(qj)
                return jobs

            def outproj_jobs(c, ao):
                """Partial out-projection of chunk c as 8 jobs."""
                q0 = c * QC
                jobs = []
                os_tiles = {}
                for qt in range(4):
                    os_tiles[qt] = osb_pool.tile([128, D], BF16, tag='os',
                                                 name='os')
                    for half in range(2):
                        def oj(qt=qt, half=half, ao=ao, q0=q0,
                               os=os_tiles[qt]):
                            pj = ps_pp.tile([128, 512], F32, tag='pp',
                                            name='pj')
                            for dd in range(4):
                                nc.tensor.matmul(
                                    pj[:, :],
                                    lhsT=ao[dd][:, qt*128:(qt+1)*128],
                                    rhs=WO[dd][:, half*512:(half+1)*512],
                                    start=(dd == 0), stop=(dd == 3))
                            nc.vector.tensor_copy(
                                os[:, half*512:(half+1)*512], pj[:, :])
                            if half == 1:   # one merged DMA per q-tile
                                nc.sync.dma_start(
                                    po_d[q0+qt*128:q0+(qt+1)*128, :],
                                    os[:, :])
                        jobs.append(oj)
                return jobs

            # chunk-0 projections run standalone (nothing to overlap yet)
            for job in proj_jobs(0):
                job()

            prev_outproj = []
            for c in range(NQC):
                q0 = c * QC
                nkb = (q0 + QC) // 128      # causal k-blocks this chunk
                QTs = qt_sets.pop(c)
                filler = (proj_jobs(c + 1) if c + 1 < NQC else [])
                filler += prev_outproj
                nslots = 4 * (nkb // 2)
                emitted = 0
                it = 0

                ao = [ao_pool.tile([128, QC], BF16, tag=f'ao{j}',
                                   name=f'ao{j}') for j in range(4)]
                for j in range(4):            # head pair (2j, 2j+1)
                    h0, h1 = 2*j, 2*j + 1
                    ot0 = ps_ot.tile([HD + 1, QC], F32, tag='ot',
                                     name='ot0')
                    ot1 = ps_ot.tile([HD + 1, QC], F32, tag='ot',
                                     name='ot1')
                    KTe, KTo = KT[j][0:64, :], KT[j][64:128, :]
                    QTe, QTo = QTs[j][0:64, :], QTs[j][64:128, :]
                    pend = None
                    for kbp in range(nkb // 2):
                        ka, kB = 2*kbp, 2*kbp + 1
                        lo_a = max(0, ka*128 - q0)
                        lo_b = max(0, kB*128 - q0)
                        s0 = ps_s.tile([128, 2*QC], F32, tag='s', name='s0')
                        s1 = ps_s.tile([128, 2*QC], F32, tag='s', name='s1')
                        pt0 = pt_pool.tile([128, 2*QC], BF16, tag='pt',
                                           name='pt0')
                        pt1 = pt_pool.tile([128, 2*QC], BF16, tag='pt',
                                           name='pt1')
                        nc.tensor.matmul(
                            s0[:, lo_a:QC],
                            lhsT=KTe[:, ka*128:(ka+1)*128],
                            rhs=QTe[:, lo_a:QC],
                            start=True, stop=True)
                        nc.tensor.matmul(
                            s0[:, QC+lo_b:2*QC],
                            lhsT=KTe[:, kB*128:(kB+1)*128],
                            rhs=QTe[:, lo_b:QC],
                            start=True, stop=True)
                        nc.scalar.activation(
                            pt0[:, lo_a:2*QC], s0[:, lo_a:2*QC], AF.Exp)
                        if pend is not None:
                            for (pk, pl, pc0), (pp0, _pp1) in pend:
                                nc.tensor.matmul(
                                    ot0[:, pl:QC],
                                    lhsT=V[pk][:, (HD+1)*h0:(HD+1)*(h0+1)],
                                    rhs=pp0[:, pc0+pl:pc0+QC],
                                    start=(pk == 0), stop=False)
                        nc.tensor.matmul(
                            s1[:, lo_a:QC],
                            lhsT=KTo[:, ka*128:(ka+1)*128],
                            rhs=QTo[:, lo_a:QC],
                            start=True, stop=True)
                        nc.tensor.matmul(
                            s1[:, QC+lo_b:2*QC],
                            lhsT=KTo[:, kB*128:(kB+1)*128],
                            rhs=QTo[:, lo_b:QC],
                            start=True, stop=True)
                        nc.scalar.activation(
                            pt1[:, lo_a:2*QC], s1[:, lo_a:2*QC], AF.Exp)
                        if pend is not None:
                            for (pk, pl, pc0), (_pp0, pp1) in pend:
                                nc.tensor.matmul(
                                    ot1[:, pl:QC],
                                    lhsT=V[pk][:, (HD+1)*h1:(HD+1)*(h1+1)],
                                    rhs=pp1[:, pc0+pl:pc0+QC],
                                    start=(pk == 0), stop=False)
                        for kx, lox, c0 in ((ka, lo_a, 0), (kB, lo_b, QC)):
                            if kx*128 >= q0:   # causal mask on diag band
                                for ptx in (pt0, pt1):
                                    nc.gpsimd.affine_select(
                                        out=ptx[:, c0+lox:c0+lox+128],
                                        in_=ptx[:, c0+lox:c0+lox+128],
                                        compare_op=ALU.is_ge, fill=0.0,
                                        base=0, channel_multiplier=-1,
                                        pattern=[[1, 128]])
                        pend = [((ka, lo_a, 0), (pt0, pt1)),
                                ((kB, lo_b, QC), (pt0, pt1))]
                        # interleave filler PE work (next chunk's
                        # projections, previous chunk's out-projection)
                        it += 1
                        want = (len(filler) * it + nslots - 1) // nslots
                        while emitted < want:
                            filler[emitted]()
                            emitted += 1
                    for (pk, pl, pc0), (pp0, pp1) in pend:
                        nc.tensor.matmul(
                            ot0[:, pl:QC],
                            lhsT=V[pk][:, (HD+1)*h0:(HD+1)*(h0+1)],
                            rhs=pp0[:, pc0+pl:pc0+QC],
                            start=(pk == 0), stop=(pk == nkb - 1))
                        nc.tensor.matmul(
                            ot1[:, pl:QC],
                            lhsT=V[pk][:, (HD+1)*h1:(HD+1)*(h1+1)],
                            rhs=pp1[:, pc0+pl:pc0+QC],
                            start=(pk == 0), stop=(pk == nkb - 1))
                    # normalize both heads of the pair
                    rp0 = sm_pool.tile([1, QC], F32, tag='rp0', bufs=2)
                    rp1 = sm_pool.tile([1, QC], F32, tag='rp1', bufs=2)
                    din0 = sm_pool.tile([1, QC], F32, tag='din0', bufs=2)
                    din1 = sm_pool.tile([1, QC], F32, tag='din1', bufs=2)
                    nc.vector.tensor_copy(din0[:, :], ot0[HD:HD+1, :])
                    nc.vector.tensor_copy(din1[:, :], ot1[HD:HD+1, :])
                    nc.vector.reciprocal_approx_fast(
                        out=rp0[:, :], in_=din0[:, :])
                    nc.vector.reciprocal_approx_fast(
                        out=rp1[:, :], in_=din1[:, :])
                    rbs0 = sm_pool.tile([HD, QC], F32, tag='rbs0', bufs=2)
                    rbs1 = sm_pool.tile([HD, QC], F32, tag='rbs1', bufs=2)
                    nc.gpsimd.partition_broadcast(rbs0[:, :], rp0[:, :])
                    nc.gpsimd.partition_broadcast(rbs1[:, :], rp1[:, :])
                    nc.vector.tensor_tensor(
                        out=ao[j][0:HD, :], in0=ot0[0:HD, :],
                        in1=rbs0[:, :], op=ALU.mult)
                    nc.vector.tensor_tensor(
                        out=ao[j][HD:128, :], in0=ot1[0:HD, :],
                        in1=rbs1[:, :], op=ALU.mult)
                while emitted < len(filler):
                    filler[emitted]()
                    emitted += 1
                prev_outproj = outproj_jobs(c, ao)

            for job in prev_outproj:    # chunk 3's out-projection
                job()

    nc.compile()
    return nc


def _get_nc():
    if 'nc' not in _nc_cache:
        _nc_cache['nc'] = _build_nc()
    return _nc_cache['nc']


def kernel(x, w_qkv, w_out, _profile=False):
    import ml_dtypes
    from concourse.bass_utils import run_bass_kernel_spmd

    BF = ml_dtypes.bfloat16
    x = np.asarray(x, dtype=np.float32)
    w_qkv = np.asarray(w_qkv, dtype=np.float32)
    w_out = np.asarray(w_out, dtype=np.float32)

    nc = _get_nc()

    scale = np.float32(1.0 / np.sqrt(HD))
    in_maps = []
    for c in range(N_CORES):
        b, hh = c // 2, c % 2
        s, e = hh * DPC, (hh + 1) * DPC
        def packw(w, nd):   # [nd*128, cols] -> [128, nd*cols] (d along free)
            cols = w.shape[1]
            return np.ascontiguousarray(
                w.reshape(nd, 128, cols).transpose(1, 0, 2)
                .reshape(128, nd * cols))

        in_maps.append({
            'xT': np.ascontiguousarray(x[b].T).astype(BF),
            'wq': packw(w_qkv[:, s:e] * scale, 8).astype(BF),
            'wk': np.ascontiguousarray(w_qkv[:, D+s:D+e]).astype(BF),
            'wv': packw(w_qkv[:, 2*D+s:2*D+e], 8).astype(BF),
            'wo': packw(w_out[s:e, :], 4).astype(BF),
        })

    res = run_bass_kernel_spmd(nc, in_maps, core_ids=list(range(N_CORES)),
                               trace=_profile)
    out = np.empty((B, T, D), np.float32)
    for b in range(B):
        out[b] = (res.results[2*b]['po'].astype(np.float32)
                  + res.results[2*b+1]['po'].astype(np.float32))
    if _profile:
        return out, res
    return out


# revision 28
# speedup vs baseline: 1.0440x; 1.0440x over previous
"""Causal multi-head attention on 8 TRN2 NeuronCores.

Sharding: core c -> (batch b = c // 2, head-half hh = c % 2).
Each core computes QKV for its 8 heads over the full sequence of its batch,
causal flash attention, and a partial out-projection using its 512 rows of
w_out. The host sums the two partials per batch (the "all-reduce" of the
tensor-parallel out projection).

Fully fused single-stream schedule: the attention iterations of q-chunk c
interleave "filler" PE jobs — the K/V/Q projections of chunk c+1 and the
out-projection of chunk c-1 — so the tensor engine never drains at chunk
boundaries and the ACT-bound attention stretches stay packed with PE work.

All matmul operands are bf16 (same PE rate as fp32r, half the DMA/SBUF
traffic; measured end-to-end rel err ~5.6e-3 vs 2e-2 budget). PSUM fp32.

Layouts (per core):
  KT[j]  [128, 2048] bf16  K^T, head pair j (head 2j rows 0:64, 2j+1 64:128)
  V[t]   [128, 520]  bf16  V token-tile t, 8 heads x (64 cols + ones col)
                           for the softmax denominator; ones via memset
  QT[j]  [128, 512]  bf16  per-q-chunk Q^T, same row split; S matmuls
                           contract K=64 on the row halves

Shapes (hardcoded): B=4, T=2048, D=1024, H=16, HD=64.
"""
import sys

for _p in ('/opt/trn_rl_repo', '/root/.axon_site/_ro/trn_rl_repo'):
    if _p not in sys.path:
        sys.path.insert(0, _p)

import numpy as np

B, T, D = 4, 2048, 1024
H, HD = 16, 64
HPC = H // 2          # heads per core = 8
DPC = HPC * HD        # out-dims per core = 512
N_CORES = 8

_nc_cache = {}


def _build_nc():
    import concourse.bacc as bacc
    import concourse.mybir as mybir
    from concourse.tile import TileContext

    F32 = mybir.dt.float32
    BF16 = mybir.dt.bfloat16
    AF = mybir.ActivationFunctionType
    ALU = mybir.AluOpType

    CH = 512              # token chunk (== QC)
    QC = 512              # query chunk
    NKB = T // 128        # 16 k-blocks
    NQC = T // QC         # 4 query chunks
    NDT = D // 128        # 8 input-dim tiles
    VW = HPC * (HD + 1)   # V tile width = 520

    nc = bacc.Bacc('TRN2', target_bir_lowering=False, debug=False)
    xT_d = nc.dram_tensor('xT', [D, T], BF16, kind='ExternalInput')
    # wq/wv/wo ship pre-packed as [128, d, cols] so each loads in ONE DMA
    # (SP-engine trigger processing is ~0.6us per DMA and crowds startup)
    wq_d = nc.dram_tensor('wq', [128, NDT * DPC], BF16, kind='ExternalInput')
    wk_d = nc.dram_tensor('wk', [D, DPC], BF16, kind='ExternalInput')
    wv_d = nc.dram_tensor('wv', [128, NDT * DPC], BF16, kind='ExternalInput')
    wo_d = nc.dram_tensor('wo', [128, 4 * D], BF16, kind='ExternalInput')
    po_d = nc.dram_tensor('po', [T, D], BF16, kind='ExternalOutput')

    with nc.allow_low_precision(reason='bf16 matmuls by design'), \
            TileContext(nc) as tc:
        with (
            tc.tile_pool(name='kt', bufs=1) as kt_pool,
            tc.tile_pool(name='vv', bufs=1) as v_pool,
            tc.tile_pool(name='small', bufs=2) as sm_pool,
            tc.tile_pool(name='wgt', bufs=1) as w_pool,
            tc.tile_pool(name='qt', bufs=2) as qt_pool,
            tc.tile_pool(name='xs', bufs=1) as x_pool,
            tc.tile_pool(name='ao', bufs=2) as ao_pool,
            tc.tile_pool(name='pt', bufs=4) as pt_pool,
            tc.tile_pool(name='osb', bufs=4) as osb_pool,
            tc.tile_pool(name='ps_s', bufs=2, space='PSUM') as ps_s,
            tc.tile_pool(name='ps_ot', bufs=2, space='PSUM') as ps_ot,
            tc.tile_pool(name='ps_pp', bufs=2, space='PSUM') as ps_pp,
        ):
            WQb = w_pool.tile([128, NDT * DPC], BF16, tag='wq', name='wqs')
            WQb_r = WQb.rearrange('p (d c) -> p d c', c=DPC)
            WK = [w_pool.tile([128, DPC], BF16, tag=f'wk{d}',
                              name=f'wks{d}') for d in range(NDT)]
            WVb = w_pool.tile([128, NDT * DPC], BF16, tag='wv', name='wvs')
            WVb_r = WVb.rearrange('p (d c) -> p d c', c=DPC)
            WOb = w_pool.tile([128, 4 * D], BF16, tag='wo', name='wos')
            WOb_r = WOb.rearrange('p (d c) -> p d c', c=D)
            WQ = [WQb_r[:, d, :] for d in range(NDT)]
            WV = [WVb_r[:, d, :] for d in range(NDT)]
            WO = [WOb_r[:, d, :] for d in range(4)]

            # pre-warm the ACT exp table so the first real exp doesn't pay
            # the ~2.7us table load
            warm = sm_pool.tile([1, 16], F32, tag='warm', bufs=1)
            warm2 = sm_pool.tile([2, 16], F32, tag='warm2', bufs=1)
            nc.vector.memset(warm[:, :], 0.0)
            nc.scalar.activation(warm[:, :], warm[:, :], AF.Exp)
            nc.gpsimd.affine_select(
                out=warm[:, :], in_=warm[:, :], compare_op=ALU.is_ge,
                fill=0.0, base=0, channel_multiplier=-1, pattern=[[1, 16]])
            nc.gpsimd.partition_broadcast(warm2[:, :], warm[:, :])

            KT = [kt_pool.tile([128, T], BF16, tag=f'kt{j}', name=f'kt{j}')
                  for j in range(4)]
            V = [v_pool.tile([128, VW], BF16, tag=f'v{t}', name=f'v{t}')
                 for t in range(NKB)]

            # x fully resident: one [128, T] tile per input-dim slice, one
            # DMA each; chunk c reads columns [c*CH, (c+1)*CH)
            XF = [x_pool.tile([128, T], BF16, tag=f'x{d}', name=f'xf{d}')
                  for d in range(NDT)]
            xs_sets = {c: [XF[d][:, c*CH:(c+1)*CH] for d in range(NDT)]
                       for c in range(NQC)}
            qt_sets = {}

            def alloc_qt(c):
                qt_sets[c] = [qt_pool.tile([128, QC], BF16, tag=f'qt{j}',
                                           name=f'qt{j}_{c & 1}')
                              for j in range(4)]
                return qt_sets[c]

            # DMA emission order is startup-critical: chunk-0 x columns
            # interleaved with WK (small first transfers), the rest after
            for d in range(NDT):
                nc.sync.dma_start(XF[d][:, 0:CH], xT_d[d*128:(d+1)*128,
                                                       0:CH])
                nc.sync.dma_start(WK[d][:, :], wk_d[d*128:(d+1)*128, :])
            nc.sync.dma_start(WVb[:, :], wv_d[:, :])
            nc.sync.dma_start(WQb[:, :], wq_d[:, :])
            for d in range(NDT):
                nc.sync.dma_start(XF[d][:, CH:T], xT_d[d*128:(d+1)*128,
                                                       CH:T])
            nc.sync.dma_start(WOb[:, :], wo_d[:, :])

            # softmax-denominator ones columns: on-chip (no descriptor spam)
            for t in range(NKB):
                vt3 = V[t].rearrange('p (h c) -> p h c', c=HD + 1)
                nc.gpsimd.memset(vt3[:, :, HD], 1.0)

            def proj_jobs(c):
                """K/V/Q projection of chunk c as 12 single-psum-tile jobs."""
                xs = xs_sets[c]
                QTs = alloc_qt(c)
                jobs = []
                for j in range(4):
                    def kj(j=j, xs=xs, c=c):
                        pp = ps_pp.tile([128, CH], F32, tag='pp',
                                        name=f'ppk{j}')
                        for d in range(NDT):
                            nc.tensor.matmul(
                                pp[:, :],
                                lhsT=WK[d][:, j*128:(j+1)*128],
                                rhs=xs[d][:, :],
                                start=(d == 0), stop=(d == NDT - 1))
                        nc.vector.tensor_copy(
                            KT[j][:, c*CH:(c+1)*CH], pp[:, :])
                    jobs.append(kj)
                for tt in range(4):
                    def vj(tt=tt, xs=xs, c=c):
                        pp = ps_pp.tile([128, DPC], F32, tag='pp',
                                        name=f'ppv{tt}')
                        for d in range(NDT):
                            nc.tensor.matmul(
                                pp[:, :],
                                lhsT=xs[d][:, tt*128:(tt+1)*128],
                                rhs=WV[d][:, :],
                                start=(d == 0), stop=(d == NDT - 1))
                        vt3 = V[c*4 + tt].rearrange('p (h c) -> p h c',
                                                    c=HD + 1)
                        nc.vector.tensor_copy(
                            vt3[:, :, 0:HD],
                            pp.rearrange('p (h c) -> p h c', c=HD))
                    jobs.append(vj)
                for j in range(4):
                    def qj(j=j, xs=xs, QTs=QTs):
                        pp = ps_pp.tile([128, QC], F32, tag='pp',
                                        name=f'ppq{j}')
                        for d in range(NDT):
                            nc.tensor.matmul(
                                pp[:, :],
                                lhsT=WQ[d][:, j*128:(j+1)*128],
                                rhs=xs[d][:, :],
                                start=(d == 0), stop=(d == NDT - 1))
                        nc.vector.tensor_copy(QTs[j][:, :], pp[:, :])
                    jobs.append(qj)
                return jobs

            def outproj_jobs(c, ao):
                """Partial out-projection of chunk c as 8 jobs."""
                q0 = c * QC
                jobs = []
                os_tiles = {}
                for qt in range(4):
                    os_tiles[qt] = osb_pool.tile([128, D], BF16, tag='os',
                                                 name='os')
                    for half in range(2):
                        def oj(qt=qt, half=half, ao=ao, q0=q0,
                               os=os_tiles[qt]):
                            pj = ps_pp.tile([128, 512], F32, tag='pp',
                                            name='pj')
                            for dd in range(4):
                                nc.tensor.matmul(
                                    pj[:, :],
                                    lhsT=ao[dd][:, qt*128:(qt+1)*128],
                                    rhs=WO[dd][:, half*512:(half+1)*512],
                                    start=(dd == 0), stop=(dd == 3))
                            nc.vector.tensor_copy(
                                os[:, half*512:(half+1)*512], pj[:, :])
                            if half == 1:   # one merged DMA per q-tile
                                nc.sync.dma_start(
                                    po_d[q0+qt*128:q0+(qt+1)*128, :],
                                    os[:, :])
                        jobs.append(oj)
                return jobs

            # chunk-0 projections run standalone (nothing to overlap yet)
            for job in proj_jobs(0):
                job()

            prev_outproj = []
            for c in range(NQC):
                q0 = c * QC
                nkb = (q0 + QC) // 128      # causal k-blocks this chunk
                QTs = qt_sets.pop(c)
                filler = (proj_jobs(c + 1) if c + 1 < NQC else [])
                filler += prev_outproj
                nslots = 4 * (nkb // 2)
                emitted = 0
                it = 0

                ao = [ao_pool.tile([128, QC], BF16, tag=f'ao{j}',
                                   name=f'ao{j}') for j in range(4)]
                if c == NQC - 1:
                    # last chunk: out-projection accumulates per pair in
                    # SBUF so only pair 3's partials remain in the tail
                    os_f = [osb_pool.tile([128, D], BF16, tag='osf',
                                          name=f'osf{qt}')
                            for qt in range(4)]
                for j in range(4):            # head pair (2j, 2j+1)
                    h0, h1 = 2*j, 2*j + 1
                    ot0 = ps_ot.tile([HD + 1, QC], F32, tag='ot',
                                     name='ot0')
                    ot1 = ps_ot.tile([HD + 1, QC], F32, tag='ot',
                                     name='ot1')
                    KTe, KTo = KT[j][0:64, :], KT[j][64:128, :]
                    QTe, QTo = QTs[j][0:64, :], QTs[j][64:128, :]
                    pend = None
                    for kbp in range(nkb // 2):
                        ka, kB = 2*kbp, 2*kbp + 1
                        lo_a = max(0, ka*128 - q0)
                        lo_b = max(0, kB*128 - q0)
                        s0 = ps_s.tile([128, 2*QC], F32, tag='s', name='s0')
                        s1 = ps_s.tile([128, 2*QC], F32, tag='s', name='s1')
                        pt0 = pt_pool.tile([128, 2*QC], BF16, tag='pt',
                                           name='pt0')
                        pt1 = pt_pool.tile([128, 2*QC], BF16, tag='pt',
                                           name='pt1')
                        nc.tensor.matmul(
                            s0[:, lo_a:QC],
                            lhsT=KTe[:, ka*128:(ka+1)*128],
                            rhs=QTe[:, lo_a:QC],
                            start=True, stop=True)
                        nc.tensor.matmul(
                            s0[:, QC+lo_b:2*QC],
                            lhsT=KTe[:, kB*128:(kB+1)*128],
                            rhs=QTe[:, lo_b:QC],
                            start=True, stop=True)
                        nc.scalar.activation(
                            pt0[:, lo_a:2*QC], s0[:, lo_a:2*QC], AF.Exp)
                        if pend is not None:
                            for (pk, pl, pc0), (pp0, _pp1) in pend:
                                nc.tensor.matmul(
                                    ot0[:, pl:QC],
                                    lhsT=V[pk][:, (HD+1)*h0:(HD+1)*(h0+1)],
                                    rhs=pp0[:, pc0+pl:pc0+QC],
                                    start=(pk == 0), stop=False)
                        nc.tensor.matmul(
                            s1[:, lo_a:QC],
                            lhsT=KTo[:, ka*128:(ka+1)*128],
                            rhs=QTo[:, lo_a:QC],
                            start=True, stop=True)
                        nc.tensor.matmul(
                            s1[:, QC+lo_b:2*QC],
                            lhsT=KTo[:, kB*128:(kB+1)*128],
                            rhs=QTo[:, lo_b:QC],
                            start=True, stop=True)
                        nc.scalar.activation(
                            pt1[:, lo_a:2*QC], s1[:, lo_a:2*QC], AF.Exp)
                        if pend is not None:
                            for (pk, pl, pc0), (_pp0, pp1) in pend:
                                nc.tensor.matmul(
                                    ot1[:, pl:QC],
                                    lhsT=V[pk][:, (HD+1)*h1:(HD+1)*(h1+1)],
                                    rhs=pp1[:, pc0+pl:pc0+QC],
                                    start=(pk == 0), stop=False)
                        for kx, lox, c0 in ((ka, lo_a, 0), (kB, lo_b, QC)):
                            if kx*128 >= q0:   # causal mask on diag band
                                for ptx in (pt0, pt1):
                                    nc.gpsimd.affine_select(
                                        out=ptx[:, c0+lox:c0+lox+128],
                                        in_=ptx[:, c0+lox:c0+lox+128],
                                        compare_op=ALU.is_ge, fill=0.0,
                                        base=0, channel_multiplier=-1,
                                        pattern=[[1, 128]])
                        pend = [((ka, lo_a, 0), (pt0, pt1)),
                                ((kB, lo_b, QC), (pt0, pt1))]
                        # interleave filler PE work (next chunk's
                        # projections, previous chunk's out-projection)
                        it += 1
                        want = (len(filler) * it + nslots - 1) // nslots
                        while emitted < want:
                            filler[emitted]()
                            emitted += 1
                    for (pk, pl, pc0), (pp0, pp1) in pend:
                        nc.tensor.matmul(
                            ot0[:, pl:QC],
                            lhsT=V[pk][:, (HD+1)*h0:(HD+1)*(h0+1)],
                            rhs=pp0[:, pc0+pl:pc0+QC],
                            start=(pk == 0), stop=(pk == nkb - 1))
                        nc.tensor.matmul(
                            ot1[:, pl:QC],
                            lhsT=V[pk][:, (HD+1)*h1:(HD+1)*(h1+1)],
                            rhs=pp1[:, pc0+pl:pc0+QC],
                            start=(pk == 0), stop=(pk == nkb - 1))
                    # normalize both heads of the pair
                    rp0 = sm_pool.tile([1, QC], F32, tag='rp0', bufs=2)
                    rp1 = sm_pool.tile([1, QC], F32, tag='rp1', bufs=2)
                    din0 = sm_pool.tile([1, QC], F32, tag='din0', bufs=2)
                    din1 = sm_pool.tile([1, QC], F32, tag='din1', bufs=2)
                    nc.vector.tensor_copy(din0[:, :], ot0[HD:HD+1, :])
                    nc.vector.tensor_copy(din1[:, :], ot1[HD:HD+1, :])
                    nc.vector.reciprocal_approx_fast(
                        out=rp0[:, :], in_=din0[:, :])
                    nc.vector.reciprocal_approx_fast(
                        out=rp1[:, :], in_=din1[:, :])
                    rbs0 = sm_pool.tile([HD, QC], F32, tag='rbs0', bufs=2)
                    rbs1 = sm_pool.tile([HD, QC], F32, tag='rbs1', bufs=2)
                    nc.gpsimd.partition_broadcast(rbs0[:, :], rp0[:, :])
                    nc.gpsimd.partition_broadcast(rbs1[:, :], rp1[:, :])
                    nc.vector.tensor_tensor(
                        out=ao[j][0:HD, :], in0=ot0[0:HD, :],
                        in1=rbs0[:, :], op=ALU.mult)
                    nc.vector.tensor_tensor(
                        out=ao[j][HD:128, :], in0=ot1[0:HD, :],
                        in1=rbs1[:, :], op=ALU.mult)
                    if c == NQC - 1:
                        # pair j's out-proj contribution, queued as filler
                        # (runs during the following pairs' attention)
                        for qt in range(4):
                            for half in range(2):
                                def pjob(qt=qt, half=half, jj=j, ao=ao,
                                         q0=q0):
                                    pj = ps_pp.tile([128, 512], F32,
                                                    tag='pp', name='pjf')
                                    nc.tensor.matmul(
                                        pj[:, :],
                                        lhsT=ao[jj][:, qt*128:(qt+1)*128],
                                        rhs=WO[jj][:,
                                                   half*512:(half+1)*512],
                                        start=True, stop=True)
                                    dst = os_f[qt][:,
                                                   half*512:(half+1)*512]
                                    if jj == 0:
                                        nc.vector.tensor_copy(dst, pj[:, :])
                                    else:
                                        nc.vector.tensor_tensor(
                                            out=dst, in0=pj[:, :], in1=dst,
                                            op=ALU.add)
                                    if jj == 3 and half == 1:
                                        nc.sync.dma_start(
                                            po_d[q0+qt*128:q0+(qt+1)*128,
                                                 :], os_f[qt][:, :])
                                filler.append(pjob)
                while emitted < len(filler):
                    filler[emitted]()
                    emitted += 1
                prev_outproj = (outproj_jobs(c, ao)
                                if c < NQC - 1 else [])

            for job in prev_outproj:    # chunk 3's out-projection
                job()

    nc.compile()
    return nc


def _get_nc():
    if 'nc' not in _nc_cache:
        _nc_cache['nc'] = _build_nc()
    return _nc_cache['nc']


def kernel(x, w_qkv, w_out, _profile=False):
    import ml_dtypes
    from concourse.bass_utils import run_bass_kernel_spmd

    BF = ml_dtypes.bfloat16
    x = np.asarray(x, dtype=np.float32)
    w_qkv = np.asarray(w_qkv, dtype=np.float32)
    w_out = np.asarray(w_out, dtype=np.float32)

    nc = _get_nc()

    scale = np.float32(1.0 / np.sqrt(HD))
    in_maps = []
    for c in range(N_CORES):
        b, hh = c // 2, c % 2
        s, e = hh * DPC, (hh + 1) * DPC
        def packw(w, nd):   # [nd*128, cols] -> [128, nd*cols] (d along free)
            cols = w.shape[1]
            return np.ascontiguousarray(
                w.reshape(nd, 128, cols).transpose(1, 0, 2)
                .reshape(128, nd * cols))

        in_maps.append({
            'xT': np.ascontiguousarray(x[b].T).astype(BF),
            'wq': packw(w_qkv[:, s:e] * scale, 8).astype(BF),
            'wk': np.ascontiguousarray(w_qkv[:, D+s:D+e]).astype(BF),
            'wv': packw(w_qkv[:, 2*D+s:2*D+e], 8).astype(BF),
            'wo': packw(w_out[s:e, :], 4).astype(BF),
        })

    res = run_bass_kernel_spmd(nc, in_maps, core_ids=list(range(N_CORES)),
                               trace=_profile)
    out = np.empty((B, T, D), np.float32)
    for b in range(B):
        out[b] = (res.results[2*b]['po'].astype(np.float32)
                  + res.results[2*b+1]['po'].astype(np.float32))
    if _profile:
        return out, res
    return out


# revision 30
# speedup vs baseline: 1.0481x; 1.0040x over previous
"""Causal multi-head attention on 8 TRN2 NeuronCores.

Sharding: core c -> (batch b = c // 2, head-half hh = c % 2).
Each core computes QKV for its 8 heads over the full sequence of its batch,
causal flash attention, and a partial out-projection using its 512 rows of
w_out. The host sums the two partials per batch (the "all-reduce" of the
tensor-parallel out projection).

Fully fused single-stream schedule: the attention iterations of q-chunk c
interleave "filler" PE jobs — the K/V/Q projections of chunk c+1 and the
out-projection of chunk c-1 — so the tensor engine never drains at chunk
boundaries and the ACT-bound attention stretches stay packed with PE work.

All matmul operands are bf16 (same PE rate as fp32r, half the DMA/SBUF
traffic; measured end-to-end rel err ~5.6e-3 vs 2e-2 budget). PSUM fp32.

Layouts (per core):
  KT[j]  [128, 2048] bf16  K^T, head pair j (head 2j rows 0:64, 2j+1 64:128)
  V[t]   [128, 520]  bf16  V token-tile t, 8 heads x (64 cols + ones col)
                           for the softmax denominator; ones via memset
  QT[j]  [128, 512]  bf16  per-q-chunk Q^T, same row split; S matmuls
                           contract K=64 on the row halves

Shapes (hardcoded): B=4, T=2048, D=1024, H=16, HD=64.
"""
import sys

for _p in ('/opt/trn_rl_repo', '/root/.axon_site/_ro/trn_rl_repo'):
    if _p not in sys.path:
        sys.path.insert(0, _p)

import numpy as np

B, T, D = 4, 2048, 1024
H, HD = 16, 64
HPC = H // 2          # heads per core = 8
DPC = HPC * HD        # out-dims per core = 512
N_CORES = 8

_nc_cache = {}


def _build_nc():
    import concourse.bacc as bacc
    import concourse.mybir as mybir
    from concourse.tile import TileContext

    F32 = mybir.dt.float32
    BF16 = mybir.dt.bfloat16
    AF = mybir.ActivationFunctionType
    ALU = mybir.AluOpType

    CH = 512              # token chunk (== QC)
    QC = 512              # query chunk
    NKB = T // 128        # 16 k-blocks
    NQC = T // QC         # 4 query chunks
    NDT = D // 128        # 8 input-dim tiles
    VW = HPC * (HD + 1)   # V tile width = 520

    nc = bacc.Bacc('TRN2', target_bir_lowering=False, debug=False)
    xT_d = nc.dram_tensor('xT', [D, T], BF16, kind='ExternalInput')
    # wq/wv/wo ship pre-packed as [128, d, cols] so each loads in ONE DMA
    # (SP-engine trigger processing is ~0.6us per DMA and crowds startup)
    wq_d = nc.dram_tensor('wq', [128, NDT * DPC], BF16, kind='ExternalInput')
    wk_d = nc.dram_tensor('wk', [D, DPC], BF16, kind='ExternalInput')
    wv_d = nc.dram_tensor('wv', [128, NDT * DPC], BF16, kind='ExternalInput')
    wo_d = nc.dram_tensor('wo', [128, 4 * D], BF16, kind='ExternalInput')
    po_d = nc.dram_tensor('po', [T, D], BF16, kind='ExternalOutput')

    with nc.allow_low_precision(reason='bf16 matmuls by design'), \
            TileContext(nc) as tc:
        with (
            tc.tile_pool(name='kt', bufs=1) as kt_pool,
            tc.tile_pool(name='vv', bufs=1) as v_pool,
            tc.tile_pool(name='small', bufs=2) as sm_pool,
            tc.tile_pool(name='wgt', bufs=1) as w_pool,
            tc.tile_pool(name='qt', bufs=2) as qt_pool,
            tc.tile_pool(name='xs', bufs=1) as x_pool,
            tc.tile_pool(name='ao', bufs=2) as ao_pool,
            tc.tile_pool(name='pt', bufs=4) as pt_pool,
            tc.tile_pool(name='osb', bufs=4) as osb_pool,
            tc.tile_pool(name='ps_s', bufs=2, space='PSUM') as ps_s,
            tc.tile_pool(name='ps_ot', bufs=2, space='PSUM') as ps_ot,
            tc.tile_pool(name='ps_pp', bufs=2, space='PSUM') as ps_pp,
        ):
            WQb = w_pool.tile([128, NDT * DPC], BF16, tag='wq', name='wqs')
            WQb_r = WQb.rearrange('p (d c) -> p d c', c=DPC)
            WK = [w_pool.tile([128, DPC], BF16, tag=f'wk{d}',
                              name=f'wks{d}') for d in range(NDT)]
            WVb = w_pool.tile([128, NDT * DPC], BF16, tag='wv', name='wvs')
            WVb_r = WVb.rearrange('p (d c) -> p d c', c=DPC)
            WOb = w_pool.tile([128, 4 * D], BF16, tag='wo', name='wos')
            WOb_r = WOb.rearrange('p (d c) -> p d c', c=D)
            WQ = [WQb_r[:, d, :] for d in range(NDT)]
            WV = [WVb_r[:, d, :] for d in range(NDT)]
            WO = [WOb_r[:, d, :] for d in range(4)]

            # pre-warm the ACT exp table so the first real exp doesn't pay
            # the ~2.7us table load
            warm = sm_pool.tile([1, 16], F32, tag='warm', bufs=1)
            warm2 = sm_pool.tile([2, 16], F32, tag='warm2', bufs=1)
            nc.vector.memset(warm[:, :], 0.0)
            nc.scalar.activation(warm[:, :], warm[:, :], AF.Exp)
            nc.gpsimd.affine_select(
                out=warm[:, :], in_=warm[:, :], compare_op=ALU.is_ge,
                fill=0.0, base=0, channel_multiplier=-1, pattern=[[1, 16]])
            nc.gpsimd.partition_broadcast(warm2[:, :], warm[:, :])

            KT = [kt_pool.tile([128, T], BF16, tag=f'kt{j}', name=f'kt{j}')
                  for j in range(4)]
            V = [v_pool.tile([128, VW], BF16, tag=f'v{t}', name=f'v{t}')
                 for t in range(NKB)]

            # x fully resident: one [128, T] tile per input-dim slice, one
            # DMA each; chunk c reads columns [c*CH, (c+1)*CH)
            XF = [x_pool.tile([128, T], BF16, tag=f'x{d}', name=f'xf{d}')
                  for d in range(NDT)]
            xs_sets = {c: [XF[d][:, c*CH:(c+1)*CH] for d in range(NDT)]
                       for c in range(NQC)}
            qt_sets = {}

            def alloc_qt(c):
                qt_sets[c] = [qt_pool.tile([128, QC], BF16, tag=f'qt{j}',
                                           name=f'qt{j}_{c & 1}')
                              for j in range(4)]
                return qt_sets[c]

            # DMA emission order is startup-critical: chunk-0 x columns
            # interleaved with WK (small first transfers), the rest after
            for d in range(NDT):
                nc.sync.dma_start(XF[d][:, 0:CH], xT_d[d*128:(d+1)*128,
                                                       0:CH])
                nc.sync.dma_start(WK[d][:, :], wk_d[d*128:(d+1)*128, :])
            nc.sync.dma_start(WVb[:, :], wv_d[:, :])
            nc.sync.dma_start(WQb[:, :], wq_d[:, :])
            for d in range(NDT):
                nc.sync.dma_start(XF[d][:, CH:T], xT_d[d*128:(d+1)*128,
                                                       CH:T])
            nc.sync.dma_start(WOb[:, :], wo_d[:, :])

            # softmax-denominator ones columns: on-chip (no descriptor spam)
            for t in range(NKB):
                vt3 = V[t].rearrange('p (h c) -> p h c', c=HD + 1)
                nc.gpsimd.memset(vt3[:, :, HD], 1.0)

            def proj_jobs(c):
                """K/V/Q projection of chunk c as 12 single-psum-tile jobs."""
                xs = xs_sets[c]
                QTs = alloc_qt(c)
                jobs = []
                for j in range(4):
                    def kj(j=j, xs=xs, c=c):
                        pp = ps_pp.tile([128, CH], F32, tag='pp',
                                        name=f'ppk{j}')
                        for d in range(NDT):
                            nc.tensor.matmul(
                                pp[:, :],
                                lhsT=WK[d][:, j*128:(j+1)*128],
                                rhs=xs[d][:, :],
                                start=(d == 0), stop=(d == NDT - 1))
                        nc.vector.tensor_copy(
                            KT[j][:, c*CH:(c+1)*CH], pp[:, :])
                    jobs.append(kj)
                for tt in range(4):
                    def vj(tt=tt, xs=xs, c=c):
                        pp = ps_pp.tile([128, DPC], F32, tag='pp',
                                        name=f'ppv{tt}')
                        for d in range(NDT):
                            nc.tensor.matmul(
                                pp[:, :],
                                lhsT=xs[d][:, tt*128:(tt+1)*128],
                                rhs=WV[d][:, :],
                                start=(d == 0), stop=(d == NDT - 1))
                        vt3 = V[c*4 + tt].rearrange('p (h c) -> p h c',
                                                    c=HD + 1)
                        nc.vector.tensor_copy(
                            vt3[:, :, 0:HD],
                            pp.rearrange('p (h c) -> p h c', c=HD))
                    jobs.append(vj)
                for j in range(4):
                    def qj(j=j, xs=xs, QTs=QTs):
                        pp = ps_pp.tile([128, QC], F32, tag='pp',
                                        name=f'ppq{j}')
                        for d in range(NDT):
                            nc.tensor.matmul(
                                pp[:, :],
                                lhsT=WQ[d][:, j*128:(j+1)*128],
                                rhs=xs[d][:, :],
                                start=(d == 0), stop=(d == NDT - 1))
                        nc.vector.tensor_copy(QTs[j][:, :], pp[:, :])
                    jobs.append(qj)
                return jobs

            def outproj_jobs(c, ao):
                """Partial out-projection of chunk c as 8 jobs."""
                q0 = c * QC
                jobs = []
                os_tiles = {}
                for qt in range(4):
                    os_tiles[qt] = osb_pool.tile([128, D], BF16, tag='os',
                                                 name='os')
                    for half in range(2):
                        def oj(qt=qt, half=half, ao=ao, q0=q0,
                               os=os_tiles[qt]):
                            pj = ps_pp.tile([128, 512], F32, tag='pp',
                                            name='pj')
                            for dd in range(4):
                                nc.tensor.matmul(
                                    pj[:, :],
                                    lhsT=ao[dd][:, qt*128:(qt+1)*128],
                                    rhs=WO[dd][:, half*512:(half+1)*512],
                                    start=(dd == 0), stop=(dd == 3))
                            nc.vector.tensor_copy(
                                os[:, half*512:(half+1)*512], pj[:, :])
                            if half == 1:   # one merged DMA per q-tile
                                nc.sync.dma_start(
                                    po_d[q0+qt*128:q0+(qt+1)*128, :],
                                    os[:, :])
                        jobs.append(oj)
                return jobs

            # chunk-0 projections run standalone (nothing to overlap yet)
            for job in proj_jobs(0):
                job()

            prev_outproj = []
            for c in range(NQC):
                q0 = c * QC
                nkb = (q0 + QC) // 128      # causal k-blocks this chunk
                QTs = qt_sets.pop(c)
                filler = (proj_jobs(c + 1) if c + 1 < NQC else [])
                filler += prev_outproj
                nslots = 4 * (nkb // 2)
                emitted = 0
                it = 0

                ao = [ao_pool.tile([128, QC], BF16, tag=f'ao{j}',
                                   name=f'ao{j}') for j in range(4)]
                if c == NQC - 1:
                    # last chunk: out-projection accumulates per pair in
                    # SBUF so only pair 3's partials remain in the tail
                    os_f = [osb_pool.tile([128, D], BF16, tag='osf',
                                          name=f'osf{qt}')
                            for qt in range(4)]
                for j in range(4):            # head pair (2j, 2j+1)
                    h0, h1 = 2*j, 2*j + 1
                    ot0 = ps_ot.tile([HD + 1, QC], F32, tag='ot',
                                     name='ot0')
                    ot1 = ps_ot.tile([HD + 1, QC], F32, tag='ot',
                                     name='ot1')
                    KTe, KTo = KT[j][0:64, :], KT[j][64:128, :]
                    QTe, QTo = QTs[j][0:64, :], QTs[j][64:128, :]
                    pend = None
                    for kbp in range(nkb // 2):
                        ka, kB = 2*kbp, 2*kbp + 1
                        lo_a = max(0, ka*128 - q0)
                        lo_b = max(0, kB*128 - q0)
                        s0 = ps_s.tile([128, 2*QC], F32, tag='s', name='s0')
                        s1 = ps_s.tile([128, 2*QC], F32, tag='s', name='s1')
                        pt0 = pt_pool.tile([128, 2*QC], BF16, tag='pt',
                                           name='pt0')
                        pt1 = pt_pool.tile([128, 2*QC], BF16, tag='pt',
                                           name='pt1')
                        nc.tensor.matmul(
                            s0[:, lo_a:QC],
                            lhsT=KTe[:, ka*128:(ka+1)*128],
                            rhs=QTe[:, lo_a:QC],
                            start=True, stop=True)
                        nc.tensor.matmul(
                            s0[:, QC+lo_b:2*QC],
                            lhsT=KTe[:, kB*128:(kB+1)*128],
                            rhs=QTe[:, lo_b:QC],
                            start=True, stop=True)
                        nc.scalar.activation(
                            pt0[:, lo_a:2*QC], s0[:, lo_a:2*QC], AF.Exp)
                        if pend is not None:
                            for (pk, pl, pc0), (pp0, _pp1) in pend:
                                nc.tensor.matmul(
                                    ot0[:, pl:QC],
                                    lhsT=V[pk][:, (HD+1)*h0:(HD+1)*(h0+1)],
                                    rhs=pp0[:, pc0+pl:pc0+QC],
                                    start=(pk == 0), stop=False)
                        nc.tensor.matmul(
                            s1[:, lo_a:QC],
                            lhsT=KTo[:, ka*128:(ka+1)*128],
                            rhs=QTo[:, lo_a:QC],
                            start=True, stop=True)
                        nc.tensor.matmul(
                            s1[:, QC+lo_b:2*QC],
                            lhsT=KTo[:, kB*128:(kB+1)*128],
                            rhs=QTo[:, lo_b:QC],
                            start=True, stop=True)
                        nc.scalar.activation(
                            pt1[:, lo_a:2*QC], s1[:, lo_a:2*QC], AF.Exp)
                        if pend is not None:
                            for (pk, pl, pc0), (_pp0, pp1) in pend:
                                nc.tensor.matmul(
                                    ot1[:, pl:QC],
                                    lhsT=V[pk][:, (HD+1)*h1:(HD+1)*(h1+1)],
                                    rhs=pp1[:, pc0+pl:pc0+QC],
                                    start=(pk == 0), stop=False)
                        for kx, lox, c0 in ((ka, lo_a, 0), (kB, lo_b, QC)):
                            if kx*128 >= q0:   # causal mask on diag band
                                for ptx in (pt0, pt1):
                                    nc.gpsimd.affine_select(
                                        out=ptx[:, c0+lox:c0+lox+128],
                                        in_=ptx[:, c0+lox:c0+lox+128],
                                        compare_op=ALU.is_ge, fill=0.0,
                                        base=0, channel_multiplier=-1,
                                        pattern=[[1, 128]])
                        pend = [((ka, lo_a, 0), (pt0, pt1)),
                                ((kB, lo_b, QC), (pt0, pt1))]
                        # interleave filler PE work (next chunk's
                        # projections, previous chunk's out-projection)
                        it += 1
                        want = (len(filler) * it + nslots - 1) // nslots
                        while emitted < want:
                            filler[emitted]()
                            emitted += 1
                    for (pk, pl, pc0), (pp0, pp1) in pend:
                        nc.tensor.matmul(
                            ot0[:, pl:QC],
                            lhsT=V[pk][:, (HD+1)*h0:(HD+1)*(h0+1)],
                            rhs=pp0[:, pc0+pl:pc0+QC],
                            start=(pk == 0), stop=(pk == nkb - 1))
                        nc.tensor.matmul(
                            ot1[:, pl:QC],
                            lhsT=V[pk][:, (HD+1)*h1:(HD+1)*(h1+1)],
                            rhs=pp1[:, pc0+pl:pc0+QC],
                            start=(pk == 0), stop=(pk == nkb - 1))
                    # normalize both heads of the pair
                    rp0 = sm_pool.tile([1, QC], F32, tag='rp0', bufs=2)
                    rp1 = sm_pool.tile([1, QC], F32, tag='rp1', bufs=2)
                    din0 = sm_pool.tile([1, QC], F32, tag='din0', bufs=2)
                    din1 = sm_pool.tile([1, QC], F32, tag='din1', bufs=2)
                    nc.vector.tensor_copy(din0[:, :], ot0[HD:HD+1, :])
                    nc.vector.tensor_copy(din1[:, :], ot1[HD:HD+1, :])
                    nc.vector.reciprocal_approx_fast(
                        out=rp0[:, :], in_=din0[:, :])
                    nc.vector.reciprocal_approx_fast(
                        out=rp1[:, :], in_=din1[:, :])
                    rbs0 = sm_pool.tile([HD, QC], F32, tag='rbs0', bufs=2)
                    rbs1 = sm_pool.tile([HD, QC], F32, tag='rbs1', bufs=2)
                    nc.gpsimd.partition_broadcast(rbs0[:, :], rp0[:, :])
                    nc.gpsimd.partition_broadcast(rbs1[:, :], rp1[:, :])
                    nc.vector.tensor_tensor(
                        out=ao[j][0:HD, :], in0=ot0[0:HD, :],
                        in1=rbs0[:, :], op=ALU.mult)
                    nc.vector.tensor_tensor(
                        out=ao[j][HD:128, :], in0=ot1[0:HD, :],
                        in1=rbs1[:, :], op=ALU.mult)
                    if c == NQC - 1:
                        # pair j's out-proj contribution, queued as filler
                        # (runs during the following pairs' attention)
                        for qt in range(4):
                            for half in range(2):
                                def pjob(qt=qt, half=half, jj=j, ao=ao,
                                         q0=q0):
                                    pj = ps_pp.tile([128, 512], F32,
                                                    tag='pp', name='pjf')
                                    nc.tensor.matmul(
                                        pj[:, :],
                                        lhsT=ao[jj][:, qt*128:(qt+1)*128],
                                        rhs=WO[jj][:,
                                                   half*512:(half+1)*512],
                                        start=True, stop=True)
                                    dst = os_f[qt][:,
                                                   half*512:(half+1)*512]
                                    if jj == 0:
                                        nc.vector.tensor_copy(dst, pj[:, :])
                                    else:
                                        nc.vector.tensor_tensor(
                                            out=dst, in0=pj[:, :], in1=dst,
                                            op=ALU.add)
                                    if jj == 3 and half == 1:
                                        nc.sync.dma_start(
                                            po_d[q0+qt*128:q0+(qt+1)*128,
                                                 :], os_f[qt][:, :])
                                filler.append(pjob)
                while emitted < len(filler):
                    filler[emitted]()
                    emitted += 1
                prev_outproj = (outproj_jobs(c, ao)
                                if c < NQC - 1 else [])

            for job in prev_outproj:    # chunk 3's out-projection
                job()

    nc.compile()
    return nc


def _get_nc():
    if 'nc' not in _nc_cache:
        _nc_cache['nc'] = _build_nc()
    return _nc_cache['nc']


def kernel(x, w_qkv, w_out, _profile=False):
    import ml_dtypes
    from concourse.bass_utils import run_bass_kernel_spmd

    BF = ml_dtypes.bfloat16
    x = np.asarray(x, dtype=np.float32)
    w_qkv = np.asarray(w_qkv, dtype=np.float32)
    w_out = np.asarray(w_out, dtype=np.float32)

    nc = _get_nc()

    scale = np.float32(1.0 / np.sqrt(HD))
    in_maps = []
    for c in range(N_CORES):
        b, hh = c // 2, c % 2
        s, e = hh * DPC, (hh + 1) * DPC
        def packw(w, nd):   # [nd*128, cols] -> [128, nd*cols] (d along free)
            cols = w.shape[1]
            return np.ascontiguousarray(
                w.reshape(nd, 128, cols).transpose(1, 0, 2)
                .reshape(128, nd * cols))

        in_maps.append({
            'xT': np.ascontiguousarray(x[b].T).astype(BF),
            'wq': packw(w_qkv[:, s:e] * scale, 8).astype(BF),
            'wk': np.ascontiguousarray(w_qkv[:, D+s:D+e]).astype(BF),
            'wv': packw(w_qkv[:, 2*D+s:2*D+e], 8).astype(BF),
            'wo': packw(w_out[s:e, :], 4).astype(BF),
        })

    res = run_bass_kernel_spmd(nc, in_maps, core_ids=list(range(N_CORES)),
                               trace=_profile)
    out = np.empty((B, T, D), np.float32)
    for b in range(B):
        out[b] = (res.results[2*b]['po'].astype(np.float32)
                  + res.results[2*b+1]['po'].astype(np.float32))
    if _profile:
        return out, res
    return out


# revision 32
# speedup vs baseline: 1.0590x; 1.0104x over previous
"""Causal multi-head attention on 8 TRN2 NeuronCores.

Sharding: core c -> (batch b = c // 2, head-half hh = c % 2).
Each core computes QKV for its 8 heads over the full sequence of its batch,
causal flash attention, and a partial out-projection using its 512 rows of
w_out. The host sums the two partials per batch (the "all-reduce" of the
tensor-parallel out projection).

Fully fused single-stream schedule: the attention iterations of q-chunk c
interleave "filler" PE jobs — the K/V/Q projections of chunk c+1 and the
out-projection of chunk c-1 — so the tensor engine never drains at chunk
boundaries and the ACT-bound attention stretches stay packed with PE work.

All matmul operands are bf16 (same PE rate as fp32r, half the DMA/SBUF
traffic; measured end-to-end rel err ~5.6e-3 vs 2e-2 budget). PSUM fp32.

Layouts (per core):
  KT[j]  [128, 2048] bf16  K^T, head pair j (head 2j rows 0:64, 2j+1 64:128)
  V[t]   [128, 520]  bf16  V token-tile t, 8 heads x (64 cols + ones col)
                           for the softmax denominator; ones via memset
  QT[j]  [128, 512]  bf16  per-q-chunk Q^T, same row split; S matmuls
                           contract K=64 on the row halves

Shapes (hardcoded): B=4, T=2048, D=1024, H=16, HD=64.
"""
import sys

for _p in ('/opt/trn_rl_repo', '/root/.axon_site/_ro/trn_rl_repo'):
    if _p not in sys.path:
        sys.path.insert(0, _p)

import numpy as np

B, T, D = 4, 2048, 1024
H, HD = 16, 64
HPC = H // 2          # heads per core = 8
DPC = HPC * HD        # out-dims per core = 512
N_CORES = 8

_nc_cache = {}


def _build_nc():
    import concourse.bacc as bacc
    import concourse.mybir as mybir
    from concourse.tile import TileContext

    F32 = mybir.dt.float32
    BF16 = mybir.dt.bfloat16
    AF = mybir.ActivationFunctionType
    ALU = mybir.AluOpType

    CH = 512              # token chunk (== QC)
    QC = 512              # query chunk
    NKB = T // 128        # 16 k-blocks
    NQC = T // QC         # 4 query chunks
    NDT = D // 128        # 8 input-dim tiles
    VW = HPC * (HD + 1)   # V tile width = 520

    nc = bacc.Bacc('TRN2', target_bir_lowering=False, debug=False)
    xT_d = nc.dram_tensor('xT', [D, T], BF16, kind='ExternalInput')
    # wq/wv/wo ship pre-packed as [128, d, cols] so each loads in ONE DMA
    # (SP-engine trigger processing is ~0.6us per DMA and crowds startup)
    wq_d = nc.dram_tensor('wq', [128, NDT * DPC], BF16, kind='ExternalInput')
    wk_d = nc.dram_tensor('wk', [D, DPC], BF16, kind='ExternalInput')
    wv_d = nc.dram_tensor('wv', [128, NDT * DPC], BF16, kind='ExternalInput')
    wo_d = nc.dram_tensor('wo', [128, 4 * D], BF16, kind='ExternalInput')
    po_d = nc.dram_tensor('po', [T, D], BF16, kind='ExternalOutput')

    with nc.allow_low_precision(reason='bf16 matmuls by design'), \
            TileContext(nc) as tc:
        with (
            tc.tile_pool(name='kt', bufs=1) as kt_pool,
            tc.tile_pool(name='vv', bufs=1) as v_pool,
            tc.tile_pool(name='small', bufs=2) as sm_pool,
            tc.tile_pool(name='wgt', bufs=1) as w_pool,
            tc.tile_pool(name='qt', bufs=2) as qt_pool,
            tc.tile_pool(name='xs', bufs=1) as x_pool,
            tc.tile_pool(name='ao', bufs=2) as ao_pool,
            tc.tile_pool(name='pt', bufs=4) as pt_pool,
            tc.tile_pool(name='osb', bufs=4) as osb_pool,
            tc.tile_pool(name='ps_s', bufs=2, space='PSUM') as ps_s,
            tc.tile_pool(name='ps_ot', bufs=2, space='PSUM') as ps_ot,
            tc.tile_pool(name='ps_pp', bufs=2, space='PSUM') as ps_pp,
        ):
            WQb = w_pool.tile([128, NDT * DPC], BF16, tag='wq', name='wqs')
            WQb_r = WQb.rearrange('p (d c) -> p d c', c=DPC)
            WK = [w_pool.tile([128, DPC], BF16, tag=f'wk{d}',
                              name=f'wks{d}') for d in range(NDT)]
            WVb = w_pool.tile([128, NDT * DPC], BF16, tag='wv', name='wvs')
            WVb_r = WVb.rearrange('p (d c) -> p d c', c=DPC)
            WOb = w_pool.tile([128, 4 * D], BF16, tag='wo', name='wos')
            WOb_r = WOb.rearrange('p (d c) -> p d c', c=D)
            WQ = [WQb_r[:, d, :] for d in range(NDT)]
            WV = [WVb_r[:, d, :] for d in range(NDT)]
            WO = [WOb_r[:, d, :] for d in range(4)]

            # pre-warm the ACT exp table so the first real exp doesn't pay
            # the ~2.7us table load
            warm = sm_pool.tile([1, 16], F32, tag='warm', bufs=1)
            warm2 = sm_pool.tile([2, 16], F32, tag='warm2', bufs=1)
            nc.vector.memset(warm[:, :], 0.0)
            nc.scalar.activation(warm[:, :], warm[:, :], AF.Exp)
            nc.gpsimd.affine_select(
                out=warm[:, :], in_=warm[:, :], compare_op=ALU.is_ge,
                fill=0.0, base=0, channel_multiplier=-1, pattern=[[1, 16]])
            nc.gpsimd.partition_broadcast(warm2[:, :], warm[:, :])

            KT = [kt_pool.tile([128, T], BF16, tag=f'kt{j}', name=f'kt{j}')
                  for j in range(4)]
            V = [v_pool.tile([128, VW], BF16, tag=f'v{t}', name=f'v{t}')
                 for t in range(NKB)]

            # x fully resident: one [128, T] tile per input-dim slice, one
            # DMA each; chunk c reads columns [c*CH, (c+1)*CH)
            XF = [x_pool.tile([128, T], BF16, tag=f'x{d}', name=f'xf{d}')
                  for d in range(NDT)]
            xs_sets = {c: [XF[d][:, c*CH:(c+1)*CH] for d in range(NDT)]
                       for c in range(NQC)}
            qt_sets = {}

            def alloc_qt(c):
                qt_sets[c] = [qt_pool.tile([128, QC], BF16, tag=f'qt{j}',
                                           name=f'qt{j}_{c & 1}')
                              for j in range(4)]
                return qt_sets[c]

            # DMA emission order is startup-critical: chunk-0 x columns
            # interleaved with WK (small first transfers), the rest after
            for d in range(NDT):
                nc.sync.dma_start(XF[d][:, 0:CH], xT_d[d*128:(d+1)*128,
                                                       0:CH])
                nc.sync.dma_start(WK[d][:, :], wk_d[d*128:(d+1)*128, :])
            nc.sync.dma_start(WVb[:, :], wv_d[:, :])
            nc.sync.dma_start(WQb[:, :], wq_d[:, :])
            for d in range(NDT):
                nc.sync.dma_start(XF[d][:, CH:T], xT_d[d*128:(d+1)*128,
                                                       CH:T])
            nc.sync.dma_start(WOb[:, :], wo_d[:, :])

            # softmax-denominator ones columns: on-chip (no descriptor spam)
            for t in range(NKB):
                vt3 = V[t].rearrange('p (h c) -> p h c', c=HD + 1)
                nc.gpsimd.memset(vt3[:, :, HD], 1.0)

            def proj_jobs(c):
                """K/V/Q projection of chunk c as 12 single-psum-tile jobs."""
                xs = xs_sets[c]
                QTs = alloc_qt(c)
                jobs = []
                for j in range(4):
                    def kj(j=j, xs=xs, c=c):
                        pp = ps_pp.tile([128, CH], F32, tag='pp',
                                        name=f'ppk{j}')
                        for d in range(NDT):
                            nc.tensor.matmul(
                                pp[:, :],
                                lhsT=WK[d][:, j*128:(j+1)*128],
                                rhs=xs[d][:, :],
                                start=(d == 0), stop=(d == NDT - 1))
                        nc.vector.tensor_copy(
                            KT[j][:, c*CH:(c+1)*CH], pp[:, :])
                    jobs.append(kj)
                for tt in range(4):
                    def vj(tt=tt, xs=xs, c=c):
                        pp = ps_pp.tile([128, DPC], F32, tag='pp',
                                        name=f'ppv{tt}')
                        for d in range(NDT):
                            nc.tensor.matmul(
                                pp[:, :],
                                lhsT=xs[d][:, tt*128:(tt+1)*128],
                                rhs=WV[d][:, :],
                                start=(d == 0), stop=(d == NDT - 1))
                        vt3 = V[c*4 + tt].rearrange('p (h c) -> p h c',
                                                    c=HD + 1)
                        nc.vector.tensor_copy(
                            vt3[:, :, 0:HD],
                            pp.rearrange('p (h c) -> p h c', c=HD))
                    jobs.append(vj)
                for j in range(4):
                    def qj(j=j, xs=xs, QTs=QTs):
                        pp = ps_pp.tile([128, QC], F32, tag='pp',
                                        name=f'ppq{j}')
                        for d in range(NDT):
                            nc.tensor.matmul(
                                pp[:, :],
                                lhsT=WQ[d][:, j*128:(j+1)*128],
                                rhs=xs[d][:, :],
                                start=(d == 0), stop=(d == NDT - 1))
                        nc.vector.tensor_copy(QTs[j][:, :], pp[:, :])
                    jobs.append(qj)
                return jobs

            def outproj_jobs(c, ao):
                """Partial out-projection of chunk c as 8 jobs."""
                q0 = c * QC
                jobs = []
                os_tiles = {}
                for qt in range(4):
                    os_tiles[qt] = osb_pool.tile([128, D], BF16, tag='os',
                                                 name='os')
                    for half in range(2):
                        def oj(qt=qt, half=half, ao=ao, q0=q0,
                               os=os_tiles[qt]):
                            pj = ps_pp.tile([128, 512], F32, tag='pp',
                                            name='pj')
                            for dd in range(4):
                                nc.tensor.matmul(
                                    pj[:, :],
                                    lhsT=ao[dd][:, qt*128:(qt+1)*128],
                                    rhs=WO[dd][:, half*512:(half+1)*512],
                                    start=(dd == 0), stop=(dd == 3))
                            nc.vector.tensor_copy(
                                os[:, half*512:(half+1)*512], pj[:, :])
                            if half == 1:   # one merged DMA per q-tile
                                nc.sync.dma_start(
                                    po_d[q0+qt*128:q0+(qt+1)*128, :],
                                    os[:, :])
                        jobs.append(oj)
                return jobs

            # chunk-0 projections run standalone (nothing to overlap yet);
            # d-outer waves so compute starts as each (x, W) tile pair
            # lands instead of after all 16 DMAs. Borrow 2 idle PSUM banks
            # from the attention pools for 4-wide waves.
            xs0 = xs_sets[0]
            QTs0 = alloc_qt(0)
            for wave in ('k', 'v', 'q'):
                pp4 = ([ps_s.tile([128, CH], F32, tag='s', name=f'p0{wave}a'),
                        ps_s.tile([128, CH], F32, tag='s', name=f'p0{wave}b'),
                        ps_pp.tile([128, CH], F32, tag='pp',
                                   name=f'p0{wave}c'),
                        ps_pp.tile([128, CH], F32, tag='pp',
                                   name=f'p0{wave}d')])
                for d in range(NDT):
                    for j in range(4):
                        if wave == 'k':
                            lhsT, rhs = WK[d][:, j*128:(j+1)*128], xs0[d]
                        elif wave == 'v':
                            lhsT = xs0[d][:, j*128:(j+1)*128]
                            rhs = WV[d]
                        else:
                            lhsT, rhs = WQ[d][:, j*128:(j+1)*128], xs0[d]
                        nc.tensor.matmul(
                            pp4[j][:, :], lhsT=lhsT, rhs=rhs,
                            start=(d == 0), stop=(d == NDT - 1))
                for j in range(4):
                    if wave == 'k':
                        nc.vector.tensor_copy(KT[j][:, 0:CH], pp4[j][:, :])
                    elif wave == 'v':
                        vt3 = V[j].rearrange('p (h c) -> p h c', c=HD + 1)
                        nc.vector.tensor_copy(
                            vt3[:, :, 0:HD],
                            pp4[j].rearrange('p (h c) -> p h c', c=HD))
                    else:
                        nc.vector.tensor_copy(QTs0[j][:, :], pp4[j][:, :])

            prev_outproj = []
            for c in range(NQC):
                q0 = c * QC
                nkb = (q0 + QC) // 128      # causal k-blocks this chunk
                QTs = qt_sets.pop(c)
                filler = (proj_jobs(c + 1) if c + 1 < NQC else [])
                filler += prev_outproj
                nslots = 4 * (nkb // 2)
                emitted = 0
                it = 0

                ao = [ao_pool.tile([128, QC], BF16, tag=f'ao{j}',
                                   name=f'ao{j}') for j in range(4)]
                if c == NQC - 1:
                    # last chunk: out-projection accumulates per pair in
                    # SBUF so only pair 3's partials remain in the tail
                    os_f = [osb_pool.tile([128, D], BF16, tag='osf',
                                          name=f'osf{qt}')
                            for qt in range(4)]
                for j in range(4):            # head pair (2j, 2j+1)
                    h0, h1 = 2*j, 2*j + 1
                    ot0 = ps_ot.tile([HD + 1, QC], F32, tag='ot',
                                     name='ot0')
                    ot1 = ps_ot.tile([HD + 1, QC], F32, tag='ot',
                                     name='ot1')
                    KTe, KTo = KT[j][0:64, :], KT[j][64:128, :]
                    QTe, QTo = QTs[j][0:64, :], QTs[j][64:128, :]
                    pend = None
                    for kbp in range(nkb // 2):
                        ka, kB = 2*kbp, 2*kbp + 1
                        lo_a = max(0, ka*128 - q0)
                        lo_b = max(0, kB*128 - q0)
                        s0 = ps_s.tile([128, 2*QC], F32, tag='s', name='s0')
                        s1 = ps_s.tile([128, 2*QC], F32, tag='s', name='s1')
                        pt0 = pt_pool.tile([128, 2*QC], BF16, tag='pt',
                                           name='pt0')
                        pt1 = pt_pool.tile([128, 2*QC], BF16, tag='pt',
                                           name='pt1')
                        nc.tensor.matmul(
                            s0[:, lo_a:QC],
                            lhsT=KTe[:, ka*128:(ka+1)*128],
                            rhs=QTe[:, lo_a:QC],
                            start=True, stop=True)
                        nc.tensor.matmul(
                            s0[:, QC+lo_b:2*QC],
                            lhsT=KTe[:, kB*128:(kB+1)*128],
                            rhs=QTe[:, lo_b:QC],
                            start=True, stop=True)
                        nc.scalar.activation(
                            pt0[:, lo_a:2*QC], s0[:, lo_a:2*QC], AF.Exp)
                        if pend is not None:
                            for (pk, pl, pc0), (pp0, _pp1) in pend:
                                nc.tensor.matmul(
                                    ot0[:, pl:QC],
                                    lhsT=V[pk][:, (HD+1)*h0:(HD+1)*(h0+1)],
                                    rhs=pp0[:, pc0+pl:pc0+QC],
                                    start=(pk == 0), stop=False)
                        nc.tensor.matmul(
                            s1[:, lo_a:QC],
                            lhsT=KTo[:, ka*128:(ka+1)*128],
                            rhs=QTo[:, lo_a:QC],
                            start=True, stop=True)
                        nc.tensor.matmul(
                            s1[:, QC+lo_b:2*QC],
                            lhsT=KTo[:, kB*128:(kB+1)*128],
                            rhs=QTo[:, lo_b:QC],
                            start=True, stop=True)
                        nc.scalar.activation(
                            pt1[:, lo_a:2*QC], s1[:, lo_a:2*QC], AF.Exp)
                        if pend is not None:
                            for (pk, pl, pc0), (_pp0, pp1) in pend:
                                nc.tensor.matmul(
                                    ot1[:, pl:QC],
                                    lhsT=V[pk][:, (HD+1)*h1:(HD+1)*(h1+1)],
                                    rhs=pp1[:, pc0+pl:pc0+QC],
                                    start=(pk == 0), stop=False)
                        for kx, lox, c0 in ((ka, lo_a, 0), (kB, lo_b, QC)):
                            if kx*128 >= q0:   # causal mask on diag band
                                for ptx in (pt0, pt1):
                                    nc.gpsimd.affine_select(
                                        out=ptx[:, c0+lox:c0+lox+128],
                                        in_=ptx[:, c0+lox:c0+lox+128],
                                        compare_op=ALU.is_ge, fill=0.0,
                                        base=0, channel_multiplier=-1,
                                        pattern=[[1, 128]])
                        pend = [((ka, lo_a, 0), (pt0, pt1)),
                                ((kB, lo_b, QC), (pt0, pt1))]
                        # interleave filler PE work (next chunk's
                        # projections, previous chunk's out-projection)
                        it += 1
                        want = (len(filler) * it + nslots - 1) // nslots
                        while emitted < want:
                            filler[emitted]()
                            emitted += 1
                    for (pk, pl, pc0), (pp0, pp1) in pend:
                        nc.tensor.matmul(
                            ot0[:, pl:QC],
                            lhsT=V[pk][:, (HD+1)*h0:(HD+1)*(h0+1)],
                            rhs=pp0[:, pc0+pl:pc0+QC],
                            start=(pk == 0), stop=(pk == nkb - 1))
                        nc.tensor.matmul(
                            ot1[:, pl:QC],
                            lhsT=V[pk][:, (HD+1)*h1:(HD+1)*(h1+1)],
                            rhs=pp1[:, pc0+pl:pc0+QC],
                            start=(pk == 0), stop=(pk == nkb - 1))
                    # normalize both heads of the pair
                    rp0 = sm_pool.tile([1, QC], F32, tag='rp0', bufs=2)
                    rp1 = sm_pool.tile([1, QC], F32, tag='rp1', bufs=2)
                    din0 = sm_pool.tile([1, QC], F32, tag='din0', bufs=2)
                    din1 = sm_pool.tile([1, QC], F32, tag='din1', bufs=2)
                    nc.vector.tensor_copy(din0[:, :], ot0[HD:HD+1, :])
                    nc.vector.tensor_copy(din1[:, :], ot1[HD:HD+1, :])
                    nc.vector.reciprocal_approx_fast(
                        out=rp0[:, :], in_=din0[:, :])
                    nc.vector.reciprocal_approx_fast(
                        out=rp1[:, :], in_=din1[:, :])
                    rbs0 = sm_pool.tile([HD, QC], F32, tag='rbs0', bufs=2)
                    rbs1 = sm_pool.tile([HD, QC], F32, tag='rbs1', bufs=2)
                    nc.gpsimd.partition_broadcast(rbs0[:, :], rp0[:, :])
                    nc.gpsimd.partition_broadcast(rbs1[:, :], rp1[:, :])
                    nc.vector.tensor_tensor(
                        out=ao[j][0:HD, :], in0=ot0[0:HD, :],
                        in1=rbs0[:, :], op=ALU.mult)
                    nc.vector.tensor_tensor(
                        out=ao[j][HD:128, :], in0=ot1[0:HD, :],
                        in1=rbs1[:, :], op=ALU.mult)
                    if c == NQC - 1:
                        # pair j's out-proj contribution, queued as filler
                        # (runs during the following pairs' attention)
                        for qt in range(4):
                            for half in range(2):
                                def pjob(qt=qt, half=half, jj=j, ao=ao,
                                         q0=q0):
                                    pj = ps_pp.tile([128, 512], F32,
                                                    tag='pp', name='pjf')
                                    nc.tensor.matmul(
                                        pj[:, :],
                                        lhsT=ao[jj][:, qt*128:(qt+1)*128],
                                        rhs=WO[jj][:,
                                                   half*512:(half+1)*512],
                                        start=True, stop=True)
                                    dst = os_f[qt][:,
                                                   half*512:(half+1)*512]
                                    if jj == 0:
                                        nc.vector.tensor_copy(dst, pj[:, :])
                                    else:
                                        nc.vector.tensor_tensor(
                                            out=dst, in0=pj[:, :], in1=dst,
                                            op=ALU.add)
                                    if jj == 3 and half == 1:
                                        nc.sync.dma_start(
                                            po_d[q0+qt*128:q0+(qt+1)*128,
                                                 :], os_f[qt][:, :])
                                filler.append(pjob)
                while emitted < len(filler):
                    filler[emitted]()
                    emitted += 1
                prev_outproj = (outproj_jobs(c, ao)
                                if c < NQC - 1 else [])

            for job in prev_outproj:    # chunk 3's out-projection
                job()

    nc.compile()
    return nc


def _get_nc():
    if 'nc' not in _nc_cache:
        _nc_cache['nc'] = _build_nc()
    return _nc_cache['nc']


def kernel(x, w_qkv, w_out, _profile=False):
    import ml_dtypes
    from concourse.bass_utils import run_bass_kernel_spmd

    BF = ml_dtypes.bfloat16
    x = np.asarray(x, dtype=np.float32)
    w_qkv = np.asarray(w_qkv, dtype=np.float32)
    w_out = np.asarray(w_out, dtype=np.float32)

    nc = _get_nc()

    scale = np.float32(1.0 / np.sqrt(HD))
    in_maps = []
    for c in range(N_CORES):
        b, hh = c // 2, c % 2
        s, e = hh * DPC, (hh + 1) * DPC
        def packw(w, nd):   # [nd*128, cols] -> [128, nd*cols] (d along free)
            cols = w.shape[1]
            return np.ascontiguousarray(
                w.reshape(nd, 128, cols).transpose(1, 0, 2)
                .reshape(128, nd * cols))

        in_maps.append({
            'xT': np.ascontiguousarray(x[b].T).astype(BF),
            'wq': packw(w_qkv[:, s:e] * scale, 8).astype(BF),
            'wk': np.ascontiguousarray(w_qkv[:, D+s:D+e]).astype(BF),
            'wv': packw(w_qkv[:, 2*D+s:2*D+e], 8).astype(BF),
            'wo': packw(w_out[s:e, :], 4).astype(BF),
        })

    res = run_bass_kernel_spmd(nc, in_maps, core_ids=list(range(N_CORES)),
                               trace=_profile)
    out = np.empty((B, T, D), np.float32)
    for b in range(B):
        out[b] = (res.results[2*b]['po'].astype(np.float32)
                  + res.results[2*b+1]['po'].astype(np.float32))
    if _profile:
        return out, res
    return out
